# revision 6
# baseline (speedup 1.0000x reference)
"""Trainium2 Bass kernel for the AugmentedBrownianFollmerSDESTL sampler.

Math (per step i, dt=ts[i+1]-ts[i], gamma=1):
    u   = MLP(x, t_i)  (129->256->256->128, gelu-tanh)
    x  += u*dt + sqrt(gamma*dt)*z_i
    a1 += sqrt(dt) * sum_d(u*z)          (accumulated on host via cumsum)
    a2 += (dt/2)   * sum_d(u*u)          (accumulated on host via cumsum)

Device layout: features on partitions, batch on the free dim, so the
three layers chain through the PE without transposes.  The t-column of
W1 is folded into a per-step bias b1eff = b1 + t*W1[128,:].  Batch is
split 8 ways across cores (512 samples each); noise is pre-transposed
on the host to [steps, dim, batch_local] and the trajectory is written
transposed and rearranged on the host afterwards.
"""

import numpy as np

import concourse.bacc as bacc
import concourse.tile as tile
from concourse import mybir
from concourse.bass_utils import run_bass_kernel_spmd

N_CORES = 8
BATCH = 4096
BL = BATCH // N_CORES  # 512 per core
DIM = 128
HID = 256
NS = 100

F32 = mybir.dt.float32
F32R = mybir.dt.float32r
GELU = mybir.ActivationFunctionType.Gelu_apprx_tanh
MULT = mybir.AluOpType.mult
ADD = mybir.AluOpType.add


def _ts():
    # bit-exact match of jnp.linspace(0.0, 1.0, NS+1, dtype=float32)
    return np.arange(NS + 1, dtype=np.float32) * np.float32(1.0 / NS)


def build(ns=NS):
    ts = _ts()
    dts = ts[1:] - ts[:-1]
    sqdts = np.sqrt(dts)

    nc = bacc.Bacc("TRN2", target_bir_lowering=False)
    noiseT = nc.dram_tensor("noiseT", [ns, DIM, BL], F32, kind="ExternalInput")
    w1x = nc.dram_tensor("w1x", [DIM, HID], F32R, kind="ExternalInput")
    w2 = nc.dram_tensor("w2", [DIM, 2 * HID], F32R, kind="ExternalInput")
    w3 = nc.dram_tensor("w3", [DIM, 2 * DIM], F32R, kind="ExternalInput")
    b1e = nc.dram_tensor("b1e", [DIM, 2 * ns], F32, kind="ExternalInput")
    b23 = nc.dram_tensor("b23", [DIM, 3], F32, kind="ExternalInput")
    outT = nc.dram_tensor("outT", [ns, DIM + 2, BL], F32, kind="ExternalOutput")

    noiseT_ap, outT_ap = noiseT.ap(), outT.ap()

    with tile.TileContext(nc) as tc:
        with (
            tc.tile_pool(name="wpool", bufs=1) as wpool,
            tc.tile_pool(name="zpool", bufs=4) as zpool,
            tc.tile_pool(name="hpool", bufs=2) as hpool,
            tc.tile_pool(name="upool", bufs=2) as upool,
            tc.tile_pool(name="pupool", bufs=2) as pupool,
            tc.tile_pool(name="xpool", bufs=2) as xpool,
            tc.tile_pool(name="ppool", bufs=1, space="PSUM") as ppool,
        ):
            w1x_sb = wpool.tile([DIM, HID], F32R)
            nc.sync.dma_start(w1x_sb[:], w1x.ap()[:])
            w2_sb = wpool.tile([DIM, 2 * HID], F32R)
            nc.sync.dma_start(w2_sb[:], w2.ap()[:])
            w3_sb = wpool.tile([DIM, 2 * DIM], F32R)
            nc.sync.dma_start(w3_sb[:], w3.ap()[:])
            b1e_sb = wpool.tile([DIM, 2 * ns], F32)
            nc.sync.dma_start(b1e_sb[:], b1e.ap()[:])
            b23_sb = wpool.tile([DIM, 3], F32)
            nc.sync.dma_start(b23_sb[:], b23.ap()[:])
            # one-hot columns: e0 = [1,0] per partition, e1 = [0,1]; used as
            # stationary operands so the two batch reductions land on
            # different partitions of one PSUM bank
            e01_sb = wpool.tile([DIM, 4], F32R)
            nc.vector.memset(e01_sb[:, 0:1].bitcast(F32), 1.0)
            nc.vector.memset(e01_sb[:, 1:2].bitcast(F32), 0.0)
            nc.vector.memset(e01_sb[:, 2:3].bitcast(F32), 0.0)
            nc.vector.memset(e01_sb[:, 3:4].bitcast(F32), 1.0)

            x = xpool.tile([DIM, BL], F32R, tag="x")
            nc.vector.memset(x[:].bitcast(F32), 0.0)

            for i in range(ns):
                z = zpool.tile([DIM, BL], F32, tag="z")
                nc.sync.dma_start(z[:], noiseT_ap[i])

                h1p = ppool.tile([DIM, 2 * BL], F32, tag="h1p")
                for j in range(2):
                    nc.tensor.matmul(
                        h1p[:, j * BL:(j + 1) * BL],
                        lhsT=w1x_sb[:, j * DIM:(j + 1) * DIM],
                        rhs=x[:],
                        start=True, stop=True,
                    )
                h1 = hpool.tile([DIM, 2 * BL], F32R, tag="h1")
                for j in range(2):
                    nc.scalar.activation(
                        h1[:, j * BL:(j + 1) * BL], h1p[:, j * BL:(j + 1) * BL],
                        GELU, bias=b1e_sb[:, 2 * i + j:2 * i + j + 1],
                    )

                h2p = ppool.tile([DIM, 2 * BL], F32, tag="h2p")
                for j in range(2):
                    for k in range(2):
                        nc.tensor.matmul(
                            h2p[:, j * BL:(j + 1) * BL],
                            lhsT=w2_sb[:, k * HID + j * DIM:k * HID + (j + 1) * DIM],
                            rhs=h1[:, k * BL:(k + 1) * BL],
                            start=(k == 0), stop=(k == 1),
                        )
                h2 = hpool.tile([DIM, 2 * BL], F32R, tag="h2")
                for j in range(2):
                    nc.scalar.activation(
                        h2[:, j * BL:(j + 1) * BL], h2p[:, j * BL:(j + 1) * BL],
                        GELU, bias=b23_sb[:, j:j + 1],
                    )

                up = ppool.tile([DIM, BL], F32, tag="up")
                for k in range(2):
                    nc.tensor.matmul(
                        up[:],
                        lhsT=w3_sb[:, k * DIM:(k + 1) * DIM],
                        rhs=h2[:, k * BL:(k + 1) * BL],
                        start=(k == 0), stop=(k == 1),
                    )
                u = upool.tile([DIM, BL], F32, tag="u")
                nc.vector.tensor_scalar(u[:], up[:], b23_sb[:, 2:3], None, op0=ADD)

                pu = pupool.tile([DIM, 2 * BL], F32R, tag="pu")
                nc.vector.tensor_mul(pu[:, :BL], u[:], z[:])
                nc.vector.tensor_mul(pu[:, BL:], u[:], u[:])

                red = ppool.tile([2, BL], F32, tag="red")
                nc.tensor.matmul(red[:], lhsT=e01_sb[:, 0:2], rhs=pu[:, :BL],
                                 start=True, stop=False)
                nc.tensor.matmul(red[:], lhsT=e01_sb[:, 2:4], rhs=pu[:, BL:],
                                 start=False, stop=True)
                redsb = upool.tile([2, BL], F32, tag="redsb")
                nc.vector.tensor_copy(redsb[:], red[:])

                xa = xpool.tile([DIM, BL], F32, tag="xa")
                nc.vector.scalar_tensor_tensor(
                    xa[:], u[:], float(dts[i]), x[:].bitcast(F32),
                    op0=MULT, op1=ADD)
                x = xpool.tile([DIM, BL], F32R, tag="x")
                nc.vector.scalar_tensor_tensor(
                    x[:], z[:], float(sqdts[i]), xa[:], op0=MULT, op1=ADD)

                nc.sync.dma_start(outT_ap[i, 0:DIM, :], x[:].bitcast(F32))
                nc.sync.dma_start(outT_ap[i, DIM:DIM + 2, :], redsb[:])

    nc.compile()
    return nc


_nc_cache = {}


def _get_nc(ns=NS):
    if ns not in _nc_cache:
        _nc_cache[ns] = build(ns)
    return _nc_cache[ns]


def _host_inputs(noise, W1, b1, W2, b2, W3, b3, ns=NS):
    ts = _ts()
    noise = np.asarray(noise, dtype=np.float32)
    W1 = np.asarray(W1, dtype=np.float32)
    b1 = np.asarray(b1, dtype=np.float32)
    W2 = np.asarray(W2, dtype=np.float32)
    b2 = np.asarray(b2, dtype=np.float32)
    W3 = np.asarray(W3, dtype=np.float32)
    b3 = np.asarray(b3, dtype=np.float32)

    w1x = np.ascontiguousarray(W1[:DIM, :])
    w2 = np.concatenate([W2[:DIM, :], W2[DIM:, :]], axis=1)
    w3 = np.concatenate([W3[:DIM, :], W3[DIM:, :]], axis=1)
    # b1eff[i] = b1 + ts[i] * W1[128, :]   -> [DIM, 2*ns] with col 2i+j
    b1eff = b1[None, :] + ts[:ns, None] * W1[DIM, :][None, :]
    b1e = np.ascontiguousarray(
        b1eff.reshape(ns, 2, DIM).transpose(2, 0, 1).reshape(DIM, 2 * ns))
    b23 = np.stack([b2[:DIM], b2[DIM:], b3], axis=1)

    # noise [ns, BATCH, DIM] -> per-core [ns, DIM, BL]
    nT = np.ascontiguousarray(
        noise[:ns].reshape(ns, N_CORES, BL, DIM).transpose(1, 0, 3, 2))

    in_maps = []
    for c in range(N_CORES):
        in_maps.append({
            "noiseT": nT[c],
            "w1x": w1x, "w2": w2, "w3": w3, "b1e": b1e, "b23": b23,
        })
    return in_maps


def _assemble(results, ns=NS):
    ts = _ts()
    dts = ts[1:ns + 1] - ts[:ns]
    sqdts = np.sqrt(dts)
    traj = np.zeros((ns + 1, BATCH, DIM + 2), dtype=np.float32)
    for c in range(N_CORES):
        o = results[c]["outT"]  # [ns, DIM+2, BL]
        sl = slice(c * BL, (c + 1) * BL)
        traj[1:, sl, :DIM] = o[:, :DIM, :].transpose(0, 2, 1)
        inc1 = sqdts[:, None] * o[:, DIM, :]
        inc2 = (o[:, DIM + 1, :] / np.float32(2.0)) * dts[:, None]
        traj[1:, sl, DIM] = np.cumsum(inc1, axis=0)
        traj[1:, sl, DIM + 1] = np.cumsum(inc2, axis=0)
    return traj, ts[:ns + 1]


def kernel(noise, W1, b1, W2, b2, W3, b3):
    nc = _get_nc()
    in_maps = _host_inputs(noise, W1, b1, W2, b2, W3, b3)
    res = run_bass_kernel_spmd(nc, in_maps, core_ids=list(range(N_CORES)))
    return _assemble(res.results)
